# revision 10
# baseline (speedup 1.0000x reference)
"""Trainium2 Bass kernel: 4D convolution (kernel 3^4, stride 1, pad 1) + bias.

  out[b,o,t,d,h,w] = bias[o] +
      sum_{i,at,ad,ah,aw} x[b,i,t+at-1,d+ad-1,h+ah-1,w+aw-1] * W[o,i,at,ad,ah,aw]

Shapes: x [2,16,8,8,32,32], W [32,16,3,3,3,3], bias [32] -> out [2,32,8,8,32,32].

Distribution (8 cores): data-parallel over the 16 (b, t) output slices, 2
adjacent t's per core.  Each core gets a 4-plane t-slab (t0-1 .. t0+2, zero
padded at tensor edges) plus replicated (host-retransformed) weights, and
produces out[b, :, t0:t0+2].

Per-core algorithm ("D-banded implicit GEMM", single accumulation pass):
  * SBUF x tile XQ: partition p = 16*d + i  (K = 128 = D x C_in, all real
    data), free = (tplane 4, h 34, w 34), host-prepadded with the zero h/w
    halo.  All spatial shifts (at, ah, aw) are pure AP base offsets; the ad
    shifts live in the banded weights.
  * 27 K-steps s = (at, ah, aw).  For each step and d-group g (d_out 4g..4g+3)
    a host-prebuilt banded weight tile BW[2s+g] [128 K, 128 M]:
        BW[.][16*d_in + i, 32*j + o] = W[o, i, at, d_in-(4g+j)+1, ah, aw]
    when d_in-(4g+j)+1 in {0,1,2} (and 0 <= d_in < 8), else 0.  A single
    fp32r matmul then contracts (i, ad) for 4 output d's at once (the zero
    band cells cost no extra time: matmul duration is N cycles regardless).
  * 8 PSUM banks = (unit u, d-group g, h-half hh), each [128, 512] fp32,
    accumulate in place over all 27 steps (start at s=0, stop at s=26).
    rhs AP = XQ[:, u+at, 16*hh+ah : +16, aw : aw+32]  -> N = 16*32 = 512.
  * float32r: 1 PE cycle/row at N >= 256 (4x faster than fp32, ~tf32
    mantissa) with fp32 input bytes reinterpreted, fp32 PSUM accumulation.
  * Evict PSUM via ScalarE activation (identity + per-partition bias), DMA
    straight out to HBM in the output layout.

The host-side input transforms (t-slab extraction + halo pad, banded weight
layout, bias broadcast) are pure data-layout work done in numpy inside
kernel(); the hardware kernel consumes them as its external inputs.
"""

import numpy as np

I_C, O_C = 16, 32
B_FULL, T_FULL, D, H, W = 2, 8, 8, 32, 32
HP, WP = H + 2, W + 2
TPL = 4  # t-planes per core: t0-1 .. t0+2
PLANE = HP * WP
XFREE = TPL * PLANE
N_CORES = 8
NSTEP = 27  # (at, ah, aw) K-steps
NBW = 2 * NSTEP  # banded tiles: (step, d-group)

_NC_CACHE: list = []


def emit_conv(tc, y_d, xq_d, bw_d, bb_d):
    """Emit the per-core conv program into TileContext `tc`.

    y_d [2, 32, 8, 32, 32] out; xq_d [128, 4, 34, 34] padded x slab;
    bw_d [54, 128, 128] banded weights; bb_d [128] broadcast bias.
    """
    import concourse.mybir as mybir

    nc = tc.nc
    f32 = mybir.dt.float32
    f32r = mybir.dt.float32r
    Ident = mybir.ActivationFunctionType.Identity
    # reinterpret the (already fp32) dram data as fp32r for the PE
    xq_d = xq_d.bitcast(f32r)
    bw_d = bw_d.bitcast(f32r)

    with (
        tc.tile_pool(name="xpool", bufs=1) as xpool,
        tc.tile_pool(name="wpool", bufs=1) as wpool,
        tc.tile_pool(name="opool", bufs=4) as opool,
        tc.tile_pool(name="ppool", bufs=1, space="PSUM") as ppool,
    ):
        # ---- weights: first chunk early so the PE can start ASAP ----
        BW = wpool.tile([128, NBW * 128], f32r, name="BW")
        BWv = BW.rearrange("p (k m) -> p k m", k=NBW)
        bw_r = bw_d.rearrange("k p m -> p k m")
        CH = 6
        chunks = list(range(0, NBW, CH))
        nc.sync.dma_start(out=BWv[:, 0:CH], in_=bw_r[:, 0:CH])

        # ---- x slab: partitions (d, i), free (tplane, h+halo, w+halo) ----
        XQ = xpool.tile([128, XFREE], f32r, name="XQ")
        XQv = XQ.rearrange("p (t h w) -> p t h w", t=TPL, h=HP, w=WP)
        for tp in range(TPL):
            nc.sync.dma_start(out=XQv[:, tp], in_=xq_d[:, tp])

        for c0 in chunks[1:]:
            nc.sync.dma_start(out=BWv[:, c0 : c0 + CH], in_=bw_r[:, c0 : c0 + CH])

        BB = wpool.tile([128, 1], f32, name="BB")
        nc.sync.dma_start(out=BB[:, :], in_=bb_d.rearrange("(p u) -> p u", u=1))

        # ---- PSUM accumulators: 8 banks = (u, g, hh) ----
        acc = {}
        for u in range(2):
            for g in range(2):
                for hh in range(2):
                    pt = ppool.tile(
                        [128, 512], f32, name=f"acc{u}{g}{hh}", tag=f"acc{u}{g}{hh}"
                    )
                    acc[u, g, hh] = pt

        # ---- main accumulation: 27 K-steps x 2 d-groups x 4 banks ----
        steps = [(at, ah, aw) for at in range(3) for ah in range(3) for aw in range(3)]
        last = len(steps) - 1
        for si, (at, ah, aw) in enumerate(steps):
            for g in range(2):
                lhsT = BWv[:, 2 * si + g, :]
                for u in range(2):
                    for hh in range(2):
                        rhs = XQv[
                            :,
                            u + at,
                            16 * hh + ah : 16 * hh + ah + 16,
                            aw : aw + W,
                        ]
                        nc.tensor.matmul(
                            out=acc[u, g, hh][:, :],
                            lhsT=lhsT,
                            rhs=rhs,
                            start=(si == 0),
                            stop=(si == last),
                        )

        # ---- evict: PSUM -> (identity + bias) -> SBUF -> HBM ----
        for u in range(2):
            for g in range(2):
                for hh in range(2):
                    ot = opool.tile([128, 512], f32, name="ot", tag="ot")
                    nc.scalar.activation(
                        ot[:, :], acc[u, g, hh][:, :], Ident, bias=BB[:, :], scale=1.0
                    )
                    ydst = y_d[
                        u, :, 4 * g : 4 * g + 4, 16 * hh : 16 * hh + 16, :
                    ].rearrange("o d h w -> d o (h w)")
                    nc.sync.dma_start(out=ydst, in_=ot[:, :])


def build_nc():
    if _NC_CACHE:
        return _NC_CACHE[0]
    import concourse.bacc as bacc
    import concourse.mybir as mybir
    from concourse.tile import TileContext

    f32 = mybir.dt.float32
    nc = bacc.Bacc("TRN2", target_bir_lowering=False, debug=False, num_devices=N_CORES)
    xq_d = nc.dram_tensor("xq", [128, TPL, HP, WP], f32, kind="ExternalInput").ap()
    bw_d = nc.dram_tensor("bw", [NBW, 128, 128], f32, kind="ExternalInput").ap()
    bb_d = nc.dram_tensor("bb", [128], f32, kind="ExternalInput").ap()
    y_d = nc.dram_tensor("y", [2, O_C, D, H, W], f32, kind="ExternalOutput").ap()
    with TileContext(nc) as tc:
        emit_conv(tc, y_d, xq_d, bw_d, bb_d)
    nc.compile()
    _NC_CACHE.append(nc)
    return nc


def build_banded_weights(weight):
    """W [32,16,3,3,3,3] -> bw [54, 128, 128] banded tiles (k = 2*step + g)."""
    bw = np.zeros((NBW, 128, 128), dtype=np.float32)
    steps = [(at, ah, aw) for at in range(3) for ah in range(3) for aw in range(3)]
    for si, (at, ah, aw) in enumerate(steps):
        for g in range(2):
            k = 2 * si + g
            for j in range(4):
                for ad in range(3):
                    d_in = 4 * g + j + ad - 1
                    if 0 <= d_in < D:
                        # [i, o] block at rows 16*d_in, cols 32*j
                        bw[
                            k,
                            16 * d_in : 16 * (d_in + 1),
                            32 * j : 32 * (j + 1),
                        ] = weight[:, :, at, ad, ah, aw].T
    return bw


def shard_inputs(x, weight, bias):
    """Full inputs -> per-core in_maps (padded t-slab, banded weights, bias)."""
    x = np.ascontiguousarray(np.asarray(x, dtype=np.float32))
    weight = np.ascontiguousarray(np.asarray(weight, dtype=np.float32))
    bias = np.ascontiguousarray(np.asarray(bias, dtype=np.float32))

    bw = build_banded_weights(weight)
    bb = np.ascontiguousarray(np.tile(bias, 4))  # partition c = 32j + o -> bias[o]

    in_maps = []
    for c in range(N_CORES):
        b = c // 4
        t0 = 2 * (c % 4)
        slab = np.zeros((I_C, TPL, D, H, W), dtype=np.float32)
        lo, hi = t0 - 1, t0 + 3
        slo, shi = max(lo, 0), min(hi, T_FULL)
        slab[:, slo - lo : shi - lo] = x[b, :, slo:shi]
        # partition p = 16*d + i, free (t, h+halo, w+halo) with zero pad
        xq = np.zeros((128, TPL, HP, WP), dtype=np.float32)
        xq[:, :, 1 : 1 + H, 1 : 1 + W] = slab.transpose(2, 0, 1, 3, 4).reshape(
            128, TPL, H, W
        )
        in_maps.append({"xq": xq, "bw": bw, "bb": bb})
    return in_maps


def unshard_outputs(results):
    out = np.empty((B_FULL, O_C, T_FULL, D, H, W), dtype=np.float32)
    for c in range(N_CORES):
        b = c // 4
        t0 = 2 * (c % 4)
        y = results[c]["y"]
        out[b, :, t0] = y[0]
        out[b, :, t0 + 1] = y[1]
    return out


def run(inputs, trace=False, **kwargs):
    from concourse.bass_utils import run_bass_kernel_spmd

    nc = build_nc()
    in_maps = shard_inputs(inputs["x"], inputs["weight"], inputs["bias"])
    res = run_bass_kernel_spmd(
        nc, in_maps, core_ids=list(range(N_CORES)), trace=trace, **kwargs
    )
    return unshard_outputs(res.results), res


def kernel(x, weight, bias):
    out, _ = run({"x": x, "weight": weight, "bias": bias})
    return out


# revision 13
# speedup vs baseline: 1.1751x; 1.1751x over previous
"""Trainium2 Bass kernel: 4D convolution (kernel 3^4, stride 1, pad 1) + bias.

  out[b,o,t,d,h,w] = bias[o] +
      sum_{i,at,ad,ah,aw} x[b,i,t+at-1,d+ad-1,h+ah-1,w+aw-1] * W[o,i,at,ad,ah,aw]

Shapes: x [2,16,8,8,32,32], W [32,16,3,3,3,3], bias [32] -> out [2,32,8,8,32,32].

Distribution (8 cores): data-parallel over the 16 (b, t) output slices, 2
adjacent t's per core.  Each core gets a 4-plane t-slab (t0-1 .. t0+2, zero
padded at tensor edges) plus replicated (host-retransformed) weights, and
produces out[b, :, t0:t0+2].

Per-core algorithm ("D-banded implicit GEMM", single accumulation pass):
  * SBUF x tile XQ: partition p = 16*d + i  (K = 128 = D x C_in, all real
    data), free = (tplane 4, h 34, w 34), host-prepadded with the zero h/w
    halo.  All spatial shifts (at, ah, aw) are pure AP base offsets; the ad
    shifts live in the banded weights.
  * 27 K-steps s = (at, ah, aw).  For each step and d-group g (d_out 4g..4g+3)
    a host-prebuilt banded weight tile BW[2s+g] [128 K, 128 M]:
        BW[.][16*d_in + i, 32*j + o] = W[o, i, at, d_in-(4g+j)+1, ah, aw]
    when d_in-(4g+j)+1 in {0,1,2} (and 0 <= d_in < 8), else 0.  A single
    fp32r matmul then contracts (i, ad) for 4 output d's at once (the zero
    band cells cost no extra time: matmul duration is N cycles regardless).
  * 8 PSUM banks = (unit u, d-group g, h-half hh), each [128, 512] fp32,
    accumulate in place over all 27 steps (start at s=0, stop at s=26).
    rhs AP = XQ[:, u+at, 16*hh+ah : +16, aw : aw+32]  -> N = 16*32 = 512.
  * float32r: 1 PE cycle/row at N >= 256 (4x faster than fp32, ~tf32
    mantissa) with fp32 input bytes reinterpreted, fp32 PSUM accumulation.
  * Evict PSUM via ScalarE activation (identity + per-partition bias), DMA
    straight out to HBM in the output layout.

The host-side input transforms (t-slab extraction + halo pad, banded weight
layout, bias broadcast) are pure data-layout work done in numpy inside
kernel(); the hardware kernel consumes them as its external inputs.
"""

import numpy as np

I_C, O_C = 16, 32
B_FULL, T_FULL, D, H, W = 2, 8, 8, 32, 32
HP, WP = H + 2, W + 2
TPL = 4  # t-planes per core: t0-1 .. t0+2
PLANE = HP * WP
XFREE = TPL * PLANE
N_CORES = 8
NSTEP = 27  # (at, ah, aw) K-steps
NBW = 2 * NSTEP  # banded tiles: (step, d-group)

_NC_CACHE: list = []


def emit_conv(tc, y_d, xq_d, bw_d, bb_d):
    """Emit the per-core conv program into TileContext `tc`.

    y_d [2, 32, 8, 32, 32] out; xq_d [128, 4, 34, 34] padded x slab;
    bw_d [54, 128, 128] banded weights; bb_d [128] broadcast bias.
    """
    import concourse.mybir as mybir

    nc = tc.nc
    f32 = mybir.dt.float32
    f32r = mybir.dt.float32r
    Ident = mybir.ActivationFunctionType.Identity
    # reinterpret the (already fp32) dram data as fp32r for the PE
    xq_d = xq_d.bitcast(f32r)
    bw_d = bw_d.bitcast(f32r)

    with (
        tc.tile_pool(name="xpool", bufs=1) as xpool,
        tc.tile_pool(name="wpool", bufs=1) as wpool,
        tc.tile_pool(name="opool", bufs=3) as opool,
        tc.tile_pool(name="ppool", bufs=1, space="PSUM") as ppool,
    ):
        # ---- PSUM accumulators: 8 banks = (u, g, hh) ----
        acc = {}
        for u in range(2):
            for g in range(2):
                for hh in range(2):
                    pt = ppool.tile(
                        [128, 512], f32, name=f"acc{u}{g}{hh}", tag=f"acc{u}{g}{hh}"
                    )
                    acc[u, g, hh] = pt

        # ---- warmup: keep the PE busy (and un-throttle HAM) during the
        # input-DMA lead-in.  Zero matmuls into bank 0; the first real
        # matmul there uses start=True, which discards these results.
        WZ = wpool.tile([128, 128], f32, name="WZ")
        nc.vector.memset(WZ[:, :], 0.0)
        for _ in range(8):
            nc.tensor.matmul(
                out=acc[0, 0, 0][:, 0:128],
                lhsT=WZ[:, :],
                rhs=WZ[:, :],
                start=True,
                stop=True,
            )

        # ---- weights (k = g*27 + si, g-major): geometric chunks so the
        # first bank's tiles land first, then x planes, then the rest ----
        BW = wpool.tile([128, NBW * 128], f32r, name="BW")
        BWv = BW.rearrange("p (k m) -> p k m", k=NBW)
        bw_r = bw_d.rearrange("k p m -> p k m")

        XQ = xpool.tile([128, XFREE], f32r, name="XQ")
        XQv = XQ.rearrange("p (t h w) -> p t h w", t=TPL, h=HP, w=WP)

        nc.sync.dma_start(out=BWv[:, 0:2], in_=bw_r[:, 0:2])
        nc.sync.dma_start(out=XQv[:, 0], in_=xq_d[:, 0])
        nc.sync.dma_start(out=BWv[:, 2:10], in_=bw_r[:, 2:10])
        nc.sync.dma_start(out=XQv[:, 1], in_=xq_d[:, 1])
        nc.sync.dma_start(out=BWv[:, 10:27], in_=bw_r[:, 10:27])
        nc.sync.dma_start(out=XQv[:, 2], in_=xq_d[:, 2])
        nc.sync.dma_start(out=XQv[:, 3], in_=xq_d[:, 3])
        nc.sync.dma_start(out=BWv[:, 27:NBW], in_=bw_r[:, 27:NBW])

        BB = wpool.tile([128, 1], f32, name="BB")
        nc.sync.dma_start(out=BB[:, :], in_=bb_d.rearrange("(p u) -> p u", u=1))

        # ---- main accumulation, bank-major: each bank's 27 K-steps run
        # consecutively so its eviction overlaps the remaining MM stream ----
        steps = [(at, ah, aw) for at in range(3) for ah in range(3) for aw in range(3)]
        last = len(steps) - 1
        for g in range(2):
            for u in range(2):
                ot = opool.tile([128, 1024], f32, name="ot", tag="ot")
                for hh in range(2):
                    for si, (at, ah, aw) in enumerate(steps):
                        rhs = XQv[
                            :,
                            u + at,
                            16 * hh + ah : 16 * hh + ah + 16,
                            aw : aw + W,
                        ]
                        nc.tensor.matmul(
                            out=acc[u, g, hh][:, :],
                            lhsT=BWv[:, g * NSTEP + si, :],
                            rhs=rhs,
                            start=(si == 0),
                            stop=(si == last),
                        )
                    # evict this bank into its half of the staging tile
                    nc.scalar.activation(
                        ot[:, 512 * hh : 512 * (hh + 1)],
                        acc[u, g, hh][:, :],
                        Ident,
                        bias=BB[:, :],
                        scale=1.0,
                    )
                ydst = y_d[u, :, 4 * g : 4 * g + 4, :, :].rearrange(
                    "o d h w -> d o (h w)"
                )
                nc.sync.dma_start(out=ydst, in_=ot[:, :])


def build_nc():
    if _NC_CACHE:
        return _NC_CACHE[0]
    import concourse.bacc as bacc
    import concourse.mybir as mybir
    from concourse.tile import TileContext

    f32 = mybir.dt.float32
    nc = bacc.Bacc("TRN2", target_bir_lowering=False, debug=False, num_devices=N_CORES)
    xq_d = nc.dram_tensor("xq", [128, TPL, HP, WP], f32, kind="ExternalInput").ap()
    bw_d = nc.dram_tensor("bw", [NBW, 128, 128], f32, kind="ExternalInput").ap()
    bb_d = nc.dram_tensor("bb", [128], f32, kind="ExternalInput").ap()
    y_d = nc.dram_tensor("y", [2, O_C, D, H, W], f32, kind="ExternalOutput").ap()
    with TileContext(nc) as tc:
        emit_conv(tc, y_d, xq_d, bw_d, bb_d)
    nc.compile()
    _NC_CACHE.append(nc)
    return nc


def build_banded_weights(weight):
    """W [32,16,3,3,3,3] -> bw [54, 128, 128] banded tiles (k = g*27 + step)."""
    bw = np.zeros((NBW, 128, 128), dtype=np.float32)
    steps = [(at, ah, aw) for at in range(3) for ah in range(3) for aw in range(3)]
    for si, (at, ah, aw) in enumerate(steps):
        for g in range(2):
            k = g * NSTEP + si
            for j in range(4):
                for ad in range(3):
                    d_in = 4 * g + j + ad - 1
                    if 0 <= d_in < D:
                        # [i, o] block at rows 16*d_in, cols 32*j
                        bw[
                            k,
                            16 * d_in : 16 * (d_in + 1),
                            32 * j : 32 * (j + 1),
                        ] = weight[:, :, at, ad, ah, aw].T
    return bw


def shard_inputs(x, weight, bias):
    """Full inputs -> per-core in_maps (padded t-slab, banded weights, bias)."""
    x = np.ascontiguousarray(np.asarray(x, dtype=np.float32))
    weight = np.ascontiguousarray(np.asarray(weight, dtype=np.float32))
    bias = np.ascontiguousarray(np.asarray(bias, dtype=np.float32))

    bw = build_banded_weights(weight)
    bb = np.ascontiguousarray(np.tile(bias, 4))  # partition c = 32j + o -> bias[o]

    in_maps = []
    for c in range(N_CORES):
        b = c // 4
        t0 = 2 * (c % 4)
        slab = np.zeros((I_C, TPL, D, H, W), dtype=np.float32)
        lo, hi = t0 - 1, t0 + 3
        slo, shi = max(lo, 0), min(hi, T_FULL)
        slab[:, slo - lo : shi - lo] = x[b, :, slo:shi]
        # partition p = 16*d + i, free (t, h+halo, w+halo) with zero pad
        xq = np.zeros((128, TPL, HP, WP), dtype=np.float32)
        xq[:, :, 1 : 1 + H, 1 : 1 + W] = slab.transpose(2, 0, 1, 3, 4).reshape(
            128, TPL, H, W
        )
        in_maps.append({"xq": xq, "bw": bw, "bb": bb})
    return in_maps


def unshard_outputs(results):
    out = np.empty((B_FULL, O_C, T_FULL, D, H, W), dtype=np.float32)
    for c in range(N_CORES):
        b = c // 4
        t0 = 2 * (c % 4)
        y = results[c]["y"]
        out[b, :, t0] = y[0]
        out[b, :, t0 + 1] = y[1]
    return out


def run(inputs, trace=False, **kwargs):
    from concourse.bass_utils import run_bass_kernel_spmd

    nc = build_nc()
    in_maps = shard_inputs(inputs["x"], inputs["weight"], inputs["bias"])
    res = run_bass_kernel_spmd(
        nc, in_maps, core_ids=list(range(N_CORES)), trace=trace, **kwargs
    )
    return unshard_outputs(res.results), res


def kernel(x, weight, bias):
    out, _ = run({"x": x, "weight": weight, "bias": bias})
    return out
